# revision 46
# baseline (speedup 1.0000x reference)
"""Trainium2 8-core tensor-parallel attention kernel (Bass/Tile).

Sharding: heads tensor-parallel across 8 cores (2 heads/core).
wq/wk/wv column-sharded by head, wo row-sharded; x replicated.
Chunked ReduceScatter (bf16) after the output projection; the host
concatenates the per-core row shards into the full output.

Fused single-phase design with software pipelining:
  iteration sc emits, finely interleaved on the PE stream,
    - QKV projections + RoPE for schunk sc,
    - attention for the unit completed by schunk sc-1,
    - o-projection (+ ReduceScatter) for the unit before that,
  so the Tensor engine always has independent matmuls in flight while
  the Activation engine works through the softmax exps.  Q/K/V stay
  SBUF-resident, softmax row-sums run on vector (bf16) with a gpsimd
  partition_all_reduce, V is transposed with one DMA-transpose per
  chain, and all bulk loads are single partition-major DMAs.

Self-contained: hardcodes B=2, S=2048, DIM=2048, NH=16, HD=128.
"""
import math

import numpy as np

B, S_FULL, DIM, NH = 2, 2048, 2048, 16
HD = 128
N_CORES = 8
HPC = NH // N_CORES          # heads per core
OC = HPC * HD                # output channels per core (256)
DT = DIM // 128              # d-tiles (16)
SC_W = 512                   # schunk width (cols of flattened seq)
RS_ROWS = 512                # rows per ReduceScatter chunk

_CACHE = {}


def _build(S):
    """Build the 8-core SPMD Bass graph for sequence length S (B=2 fixed)."""
    import concourse.bass as bass
    import concourse.mybir as mybir
    import concourse.tile as tile
    from concourse import bacc
    from concourse import bass_isa

    fp32 = mybir.dt.float32
    bf16 = mybir.dt.bfloat16
    Exp = mybir.ActivationFunctionType.Exp
    Copy = mybir.ActivationFunctionType.Copy
    ADD = mybir.AluOpType.add
    RADD = bass_isa.ReduceOp.add

    FLAT = B * S                 # flattened rows (4096)
    NSC = FLAT // SC_W           # schunks (8)
    NQT = S // 128               # q/k tiles per batch (16)
    NQG = NQT // 4               # 512-col q-groups per batch (4)
    NCH = FLAT // RS_ROWS        # ReduceScatter chunks (8)
    SCALE = 1.0 / math.sqrt(HD)
    rg = [list(range(N_CORES))]

    nc = bacc.Bacc("TRN2", target_bir_lowering=False, debug=False,
                   num_devices=N_CORES)

    # ---- external parameters (partition-major for big-descriptor DMAs) ----
    xch_d = nc.declare_dram_parameter("xch", [NSC, 128, DT * SC_W], bf16,
                                      isOutput=False)
    wqp_d = nc.declare_dram_parameter("wqp", [128, DT * OC], bf16, isOutput=False)
    wkp_d = nc.declare_dram_parameter("wkp", [128, DT * OC], bf16, isOutput=False)
    wvp_d = nc.declare_dram_parameter("wvp", [128, DT * OC], bf16, isOutput=False)
    wop_d = nc.declare_dram_parameter("wop", [128, HPC * DIM], bf16, isOutput=False)
    cos_d = nc.declare_dram_parameter("cos_t", [HD, S], bf16, isOutput=False)
    sin_d = nc.declare_dram_parameter("sin_t", [HD, S], bf16, isOutput=False)
    mdg_d = nc.declare_dram_parameter("mdgp", [128, NQT * 128], fp32, isOutput=False)
    rot_d = nc.declare_dram_parameter("rotp", [128, 128], bf16, isOutput=False)
    one_d = nc.declare_dram_parameter("ones_bf", [128, 1], bf16, isOutput=False)
    onr_d = nc.declare_dram_parameter("ones_row", [1, 128], bf16, isOutput=False)
    out_d = nc.declare_dram_parameter("out", [FLAT // N_CORES, DIM], bf16,
                                      isOutput=True)

    # ---- internal DRAM (o-proj partials + RS outputs) ----
    par_d = [nc.dram_tensor(f"partial_dram{c}", [RS_ROWS, DIM], bf16)
             for c in range(NCH)]
    rs_d = [nc.dram_tensor(f"rs_out{c}", [RS_ROWS // N_CORES, DIM], bf16)
            for c in range(NCH)]

    from contextlib import ExitStack
    with tile.TileContext(nc) as tc:
        with ExitStack() as _stk:
            cpool = _stk.enter_context(tc.tile_pool(name="consts", bufs=1))
            qkvres = _stk.enter_context(tc.tile_pool(name="qkvres", bufs=1))
            xpool = _stk.enter_context(tc.tile_pool(name="xT", bufs=2))
            spool = _stk.enter_context(tc.tile_pool(name="cops", bufs=8))
            ptpool = _stk.enter_context(tc.tile_pool(name="probsT", bufs=5))
            accpool = _stk.enter_context(tc.tile_pool(name="accs", bufs=1))
            smpool = _stk.enter_context(tc.tile_pool(name="small", bufs=2))
            opool = _stk.enter_context(tc.tile_pool(name="outT", bufs=2))
            papool = _stk.enter_context(tc.tile_pool(name="partial", bufs=4))
            qkvps = _stk.enter_context(
                tc.tile_pool(name="qkvps", bufs=2, space="PSUM"))
            scps = _stk.enter_context(
                tc.tile_pool(name="scps", bufs=2, space="PSUM"))
            wkps = _stk.enter_context(
                tc.tile_pool(name="wkps", bufs=2, space="PSUM"))
            pops = _stk.enter_context(
                tc.tile_pool(name="pops", bufs=1, space="PSUM"))

            # ---- consts (gpsimd queue) ----
            wot_sb = cpool.tile([128, HPC, DIM], bf16)
            nc.gpsimd.dma_start(
                wot_sb[:], wop_d[:].rearrange("p (h e) -> p h e", h=HPC))
            cos_sb = cpool.tile([HD, S], bf16)
            nc.gpsimd.dma_start(cos_sb[:], cos_d[:])
            sin_sb = cpool.tile([HD, S], bf16)
            nc.gpsimd.dma_start(sin_sb[:], sin_d[:])
            mdg_sb = cpool.tile([128, NQT, 128], fp32)
            nc.gpsimd.dma_start(
                mdg_sb[:], mdg_d[:].rearrange("p (t k) -> p t k", t=NQT))
            rot_sb = cpool.tile([128, 128], bf16)
            nc.gpsimd.dma_start(rot_sb[:], rot_d[:])
            one_sb = cpool.tile([128, 1], bf16)
            nc.gpsimd.dma_start(one_sb[:], one_d[:])
            onr_sb = cpool.tile([1, 128], bf16)
            nc.gpsimd.dma_start(onr_sb[:], onr_d[:])

            # ---- weights + x chunks: one partition-major DMA each ----
            w_sb = {}
            xts = {}

            def load_x(sc, split=1):
                xt = xpool.tile([128, DT, SC_W], bf16, tag="xt", name=f"xt{sc}")
                step = DT // split
                for j in range(split):
                    nc.sync.dma_start(
                        xt[:, j * step:(j + 1) * step, :],
                        xch_d[sc, :, j * step * SC_W:(j + 1) * step * SC_W]
                        .rearrange("p (t c) -> p t c", t=step))
                xts[sc] = xt

            # prologue: both DGE queues, finest pieces first, in the order
            # the chains consume them (v chains first, then q, then k).
            def w_piece(nm, src, j, step, eng):
                eng.dma_start(
                    w_sb[nm][:, j * step:(j + 1) * step, :],
                    src[:, j * step * OC:(j + 1) * step * OC]
                    .rearrange("p (t e) -> p t e", t=step))

            for nm, src in (("v", wvp_d), ("q", wqp_d), ("k", wkp_d)):
                w_sb[nm] = qkvres.tile([128, DT, OC], bf16, tag=f"w{nm}",
                                       name=f"w{nm}")
            # sync queue: wv + x0 interleaved, then x1
            w_piece("v", wvp_d, 0, 4, nc.sync)
            xt0 = xpool.tile([128, DT, SC_W], bf16, tag="xt", name="xt0")
            xts[0] = xt0
            for j in range(4):
                nc.sync.dma_start(
                    xt0[:, j * 4:(j + 1) * 4, :],
                    xch_d[0, :, j * 4 * SC_W:(j + 1) * 4 * SC_W]
                    .rearrange("p (t c) -> p t c", t=4))
                if j < 3:
                    w_piece("v", wvp_d, j + 1, 4, nc.sync)
            # scalar queue: wq quartered; wk rides the gpsimd SWDGE queue
            # (third parallel path, nothing else queued there at startup)
            for j in range(4):
                w_piece("q", wqp_d, j, 4, nc.scalar)
            for j in range(4):
                w_piece("k", wkp_d, j, 4, nc.gpsimd)

            # ---- SBUF-resident q/k/v per (batch, head) ----
            qT = {(b, h): qkvres.tile([128, S], bf16, tag=f"qT{b}{h}", name=f"qT{b}{h}")
                  for b in range(B) for h in range(HPC)}
            kT = {(b, h): qkvres.tile([128, S], bf16, tag=f"kT{b}{h}", name=f"kT{b}{h}")
                  for b in range(B) for h in range(HPC)}
            vN = {(b, h): qkvres.tile([128, NQT, HD], bf16, tag=f"vN{b}{h}",
                                      name=f"vN{b}{h}")
                  for b in range(B) for h in range(HPC)}

            oT_of = {}

            # =========== emission generators ===========

            def _emit_rope(bb, s0, t, h, til):
                rp = wkps.tile([128, SC_W], fp32, tag="wk", name=f"rot{t}{h}")
                nc.tensor.matmul(rp[:], rot_sb[:], til[:], start=True, stop=True)
                dst = qT[(bb, h)] if t == "q" else kT[(bb, h)]
                t1 = spool.tile([128, SC_W], bf16, tag="t1", name=f"t1{t}{h}")
                nc.vector.tensor_mul(t1[:], til[:], cos_sb[:, s0:s0 + SC_W])
                hat = spool.tile([128, SC_W], bf16, tag="hat", name=f"hat{t}{h}")
                nc.vector.tensor_mul(hat[:], rp[:], sin_sb[:, s0:s0 + SC_W])
                nc.vector.tensor_add(dst[:, s0:s0 + SC_W], hat[:], t1[:])

            def chunk_gen(sc):
                """QKV projections + RoPE for one schunk; yields per chain."""
                bb, c0 = divmod(sc * SC_W, S)
                s0 = c0
                chains = [(t, h) for t in ("v", "q", "k") for h in range(HPC)]
                pend = []
                for ci, (t, h) in enumerate(chains):
                    ps = qkvps.tile([128, SC_W], fp32, tag="qkv", name=f"ps_{t}{h}")
                    for dt in range(DT):
                        nc.tensor.matmul(
                            ps[:],
                            w_sb[t][:, dt, h * HD:(h + 1) * HD],
                            xts[sc][:, dt, :],
                            start=(dt == 0), stop=(dt == DT - 1))
                    if t in ("q", "k"):
                        til = spool.tile([128, SC_W], bf16, tag="til",
                                         name=f"til{t}{h}")
                        nc.scalar.activation(til[:], ps[:], Copy,
                                             scale=SCALE if t == "q" else 1.0)
                        pend.append((t, h, til))
                    else:
                        vb = spool.tile([128, SC_W], bf16, tag="vb", name=f"vb{h}")
                        nc.scalar.copy(vb[:], ps[:])
                        kt0 = c0 // 128
                        nc.sync.dma_start_transpose(
                            vN[(bb, h)][:, kt0:kt0 + 4, :], vb[:])
                    if len(pend) > 1:
                        _emit_rope(bb, s0, *pend.pop(0))
                    yield
                for args in pend:
                    _emit_rope(bb, s0, *args)

            def attn_gen(bb, qg):
                """Attention for 512 q-cols (group qg); yields per kt step."""
                kmax = qg * 4 + 3
                po = {h: pops.tile([128, 512], fp32, tag=f"po{h}", name=f"po{h}")
                      for h in range(HPC)}
                acc_v = {h: accpool.tile([128, 512], bf16, tag=f"av{h}",
                                         name=f"accv{h}") for h in range(HPC)}
                acc_g = {h: accpool.tile([128, 512], bf16, tag=f"ag{h}",
                                         name=f"accg{h}") for h in range(HPC)}
                pt_hist = {h: {} for h in range(HPC)}

                def rowsum(h, kt, qlo, n):
                    pt = pt_hist[h][kt]
                    if qg == 0:
                        if kt == 0:
                            nc.vector.tensor_copy(acc_v[h][:], pt[:, :n])
                        else:
                            nc.vector.tensor_add(acc_v[h][:, qlo:512],
                                                 acc_v[h][:, qlo:512], pt[:, :n])
                        return
                    if kt == 0:
                        nc.vector.tensor_copy(acc_v[h][:], pt[:, :n])
                    elif kt == 1:
                        nc.vector.tensor_copy(acc_g[h][:], pt[:, :n])
                    elif kt % 2 == 0:
                        nc.vector.tensor_add(acc_v[h][:, qlo:512],
                                             acc_v[h][:, qlo:512], pt[:, :n])
                    else:
                        nc.vector.tensor_add(acc_g[h][:, qlo:512],
                                             acc_g[h][:, qlo:512], pt[:, :n])

                for kt in range(kmax + 1):
                    qlo = max(0, kt - qg * 4) * 128
                    n = 512 - qlo
                    for h in range(HPC):
                        sp = scps.tile([128, 512], fp32, tag="sc", name="sp")
                        nc.tensor.matmul(
                            sp[:, :n],
                            kT[(bb, h)][:, kt * 128:(kt + 1) * 128],
                            qT[(bb, h)][:, qg * 512 + qlo:(qg + 1) * 512],
                            start=True, stop=True)
                        if kt >= qg * 4:  # diagonal block: causal mask
                            nc.vector.tensor_add(
                                sp[:, 0:128], sp[:, 0:128], mdg_sb[:, kt, :])
                        pt = ptpool.tile([128, 512], bf16, tag=f"pT{h}",
                                         name=f"pT{h}")
                        pt_hist[h][kt] = pt
                        nc.scalar.activation(pt[:, :n], sp[:, :n], Exp)
                        rowsum(h, kt, qlo, n)
                    if kt >= 1:
                        kl = kt - 1
                        ql2 = max(0, kl - qg * 4) * 128
                        n2 = 512 - ql2
                        for h in range(HPC):
                            nc.tensor.matmul(
                                po[h][:, ql2:512], vN[(bb, h)][:, kl, :],
                                pt_hist[h][kl][:, :n2],
                                start=(kl == 0), stop=False)
                    yield
                for h in range(HPC):
                    nc.tensor.matmul(
                        po[h][:, 384:512], vN[(bb, h)][:, kmax, :],
                        pt_hist[h][kmax][:, :128], start=False, stop=True)

                # softmax denominators: two cheap PE matmuls (partition
                # reduce via ones, then partition broadcast) -- gpsimd
                # stays dedicated to collectives so a blocking RS trigger
                # can't delay the normalize chain.
                oT_of[(bb, qg)] = {}
                for h in range(HPC):
                    if qg > 0:
                        nc.vector.tensor_add(acc_v[h][:], acc_v[h][:],
                                             acc_g[h][:])
                    srow_ps = scps.tile([1, 512], fp32, tag="sc", name="srow_ps")
                    nc.tensor.matmul(srow_ps[:], one_sb[:], acc_v[h][:],
                                     start=True, stop=True)
                    srow = smpool.tile([1, 512], bf16, tag="srow", name="srow")
                    nc.scalar.copy(srow[:], srow_ps[:])
                    sbc_ps = scps.tile([128, 512], fp32, tag="sc", name="sbc_ps")
                    nc.tensor.matmul(sbc_ps[:], onr_sb[:], srow[:],
                                     start=True, stop=True)
                    rbc = smpool.tile([128, 512], fp32, tag="rbc", name="rbc")
                    nc.vector.reciprocal_approx_fast(rbc[:], sbc_ps[:])
                    ot = opool.tile([128, 512], bf16, tag=f"oT{h}", name=f"oT{h}")
                    nc.vector.tensor_mul(ot[:], po[h][:], rbc[:])
                    oT_of[(bb, qg)][h] = ot

            def oproj_gen(bb, qg):
                """O-projection + ReduceScatter for one unit; yields per st."""
                ot = oT_of.pop((bb, qg))
                chx = bb * NQG + qg
                for st in range(4):
                    par = papool.tile([128, DIM], bf16, tag="par", name="par")
                    for ec in range(4):
                        pp = wkps.tile([128, 512], fp32, tag="wk", name="pp")
                        for h in range(HPC):
                            nc.tensor.matmul(
                                pp[:],
                                ot[h][:, st * 128:(st + 1) * 128],
                                wot_sb[:, h, ec * 512:(ec + 1) * 512],
                                start=(h == 0), stop=(h == HPC - 1))
                        if ec % 2 == 0:
                            nc.scalar.copy(par[:, ec * 512:(ec + 1) * 512], pp[:])
                        else:
                            nc.vector.tensor_copy(
                                par[:, ec * 512:(ec + 1) * 512], pp[:])
                    nc.sync.dma_start(par_d[chx][st * 128:(st + 1) * 128, :],
                                      par[:])
                    if st < 3:
                        yield
                nc.gpsimd.collective_compute(
                    "ReduceScatter", ADD, replica_groups=rg,
                    ins=[par_d[chx][:]], outs=[rs_d[chx][:]])

            def pump(gen, n):
                """Advance gen up to n steps; return True when exhausted."""
                if gen is None:
                    return True
                for _ in range(n):
                    if next(gen, _DONE) is _DONE:
                        return True
                return False

            _DONE = object()

            def drain(ag, ogs):
                a_done = ag is None
                ogs = list(ogs)
                while not (a_done and not ogs):
                    if not a_done:
                        a_done = pump(ag, 2)
                    for og in list(ogs):
                        if pump(og, 1):
                            ogs.remove(og)

            # =========== main schedule ===========
            # iteration sc: chunk(sc) ⊗ attn(unit sc-1) ⊗ oproj(unit sc-2)
            units = [(s // NQG, s % NQG) for s in range(NSC)]
            for sc in range(NSC):
                if sc + 1 < NSC:
                    load_x(sc + 1)
                cg = chunk_gen(sc)
                ag = attn_gen(*units[sc - 1]) if sc >= 1 else None
                ogs = [oproj_gen(*units[sc - 2])] if sc >= 2 else []
                a_steps = (units[sc - 1][1] * 4 + 4) if ag else 0
                done_a = ag is None
                for ci in range(6):
                    pump(cg, 1)
                    for og in list(ogs):
                        if ci >= 1 and pump(og, 1):
                            ogs.remove(og)
                    if not done_a:
                        share = max(1, -(-a_steps // 6))
                        done_a = pump(ag, share)
                pump(cg, 1)  # run the trailing rope flush
                drain(None if done_a else ag, ogs)
            # post-loop: attn of last unit ⊗ oproj of second-to-last
            drain(attn_gen(*units[NSC - 1]), [oproj_gen(*units[NSC - 2])])
            drain(None, [oproj_gen(*units[NSC - 1])])
            # rs -> out copies: chunks 0..6 long done (no queue blocking);
            # chunk 7's copy rides the RS tail.
            for chx in range(NCH):
                nc.gpsimd.dma_start(out_d[chx * 64:(chx + 1) * 64, :],
                                    rs_d[chx][:])

    nc.compile()
    return nc


def _get_nc(S):
    if S not in _CACHE:
        _CACHE[S] = _build(S)
    return _CACHE[S]


def make_inputs(x, freqs_cis, mask, wq, wk, wv, wo):
    """Host-side sharding / layout prep. Returns in_maps for 8 cores."""
    import ml_dtypes
    bf = ml_dtypes.bfloat16
    S = x.shape[1]
    FLAT = B * S
    NSC = FLAT // SC_W

    flat_xt = np.asarray(x, np.float32).reshape(FLAT, DIM).T  # [DIM, FLAT]
    # chunk-partition-major: xch[sc, p, t*SC_W + c] = xt[t*128+p, sc*SC_W+c]
    xch = np.ascontiguousarray(
        flat_xt.reshape(DT, 128, FLAT)[:, :, :]
        .transpose(1, 0, 2)            # [128, DT, FLAT]
        .reshape(128, DT, NSC, SC_W)
        .transpose(2, 0, 1, 3)         # [NSC, 128, DT, SC_W]
        .reshape(NSC, 128, DT * SC_W)).astype(bf)

    cos = np.asarray(freqs_cis[..., 0], np.float32)   # [S, HD/2]
    sin = np.asarray(freqs_cis[..., 1], np.float32)
    cos_t = np.ascontiguousarray(np.repeat(cos.T, 2, axis=0)).astype(bf)
    sin_t = np.ascontiguousarray(np.repeat(sin.T, 2, axis=0)).astype(bf)
    m = np.asarray(mask, np.float32)[0, 0]
    nqt = S // 128
    mask_diag = np.stack([m[i * 128:(i + 1) * 128, i * 128:(i + 1) * 128].T
                          for i in range(nqt)])       # [NQT, 128, 128]
    mdgp = np.ascontiguousarray(
        mask_diag.transpose(1, 0, 2).reshape(128, nqt * 128))

    P = np.zeros((128, 128), np.float32)
    for j in range(64):
        P[2 * j, 2 * j + 1] = -1.0
        P[2 * j + 1, 2 * j] = 1.0
    rotp = np.ascontiguousarray(P.T).astype(bf)

    def pmajor(w):  # [DIM, OC] -> [128, DT*OC]
        return np.ascontiguousarray(
            w.reshape(DT, 128, -1).transpose(1, 0, 2).reshape(128, -1))

    in_maps = []
    for c in range(N_CORES):
        r = slice(c * OC, (c + 1) * OC)
        wqt = np.asarray(wq, np.float32)[r, :].T      # [DIM, OC]
        wkt = np.asarray(wk, np.float32)[r, :].T
        wvt = np.asarray(wv, np.float32)[r, :].T
        wot = np.asarray(wo, np.float32)[:, r].T      # [OC, DIM]
        wop = np.ascontiguousarray(
            wot.reshape(HPC, 128, DIM).transpose(1, 0, 2).reshape(128, -1))
        in_maps.append({
            "xch": xch,
            "wqp": pmajor(wqt).astype(bf),
            "wkp": pmajor(wkt).astype(bf),
            "wvp": pmajor(wvt).astype(bf),
            "wop": wop.astype(bf),
            "cos_t": cos_t,
            "sin_t": sin_t,
            "mdgp": mdgp,
            "rotp": rotp,
            "ones_bf": np.ones((128, 1), dtype=bf),
            "ones_row": np.ones((1, 128), dtype=bf),
        })
    return in_maps


def assemble(results, S):
    """Undo the per-core ReduceScatter sharding into the full output."""
    nch = B * S // RS_ROWS
    full = np.empty((B * S, DIM), np.float32)
    for c in range(N_CORES):
        o = np.asarray(results[c]["out"], np.float32)  # [512, DIM]
        for chx in range(nch):
            full[chx * 512 + c * 64:chx * 512 + (c + 1) * 64] = \
                o[chx * 64:(chx + 1) * 64]
    return full.reshape(B, S, DIM)


def kernel(x, start_pos, freqs_cis, mask, wq, wk, wv, wo):
    from concourse.bass_utils import run_bass_kernel_spmd
    S = x.shape[1]
    nc = _get_nc(S)
    in_maps = make_inputs(x, freqs_cis, mask, wq, wk, wv, wo)
    res = run_bass_kernel_spmd(nc, in_maps, core_ids=list(range(N_CORES)))
    return assemble(res.results, S)


# revision 47
# speedup vs baseline: 1.0265x; 1.0265x over previous
"""Trainium2 8-core tensor-parallel attention kernel (Bass/Tile).

Sharding: heads tensor-parallel across 8 cores (2 heads/core).
wq/wk/wv column-sharded by head, wo row-sharded; x replicated.
Chunked ReduceScatter (bf16) after the output projection; the host
concatenates the per-core row shards into the full output.

Fused single-phase design with software pipelining:
  iteration sc emits, finely interleaved on the PE stream,
    - QKV projections + RoPE for schunk sc,
    - attention for the unit completed by schunk sc-1,
    - o-projection (+ ReduceScatter) for the unit before that,
  so the Tensor engine always has independent matmuls in flight while
  the Activation engine works through the softmax exps.  Q/K/V stay
  SBUF-resident, softmax row-sums run on vector (bf16) with a gpsimd
  partition_all_reduce, V is transposed with one DMA-transpose per
  chain, and all bulk loads are single partition-major DMAs.

Self-contained: hardcodes B=2, S=2048, DIM=2048, NH=16, HD=128.
"""
import math

import numpy as np

B, S_FULL, DIM, NH = 2, 2048, 2048, 16
HD = 128
N_CORES = 8
HPC = NH // N_CORES          # heads per core
OC = HPC * HD                # output channels per core (256)
DT = DIM // 128              # d-tiles (16)
SC_W = 512                   # schunk width (cols of flattened seq)
RS_ROWS = 512                # rows per ReduceScatter chunk

_CACHE = {}


def _build(S):
    """Build the 8-core SPMD Bass graph for sequence length S (B=2 fixed)."""
    import concourse.bass as bass
    import concourse.mybir as mybir
    import concourse.tile as tile
    from concourse import bacc
    from concourse import bass_isa

    fp32 = mybir.dt.float32
    bf16 = mybir.dt.bfloat16
    Exp = mybir.ActivationFunctionType.Exp
    Copy = mybir.ActivationFunctionType.Copy
    ADD = mybir.AluOpType.add
    RADD = bass_isa.ReduceOp.add

    FLAT = B * S                 # flattened rows (4096)
    NSC = FLAT // SC_W           # schunks (8)
    NQT = S // 128               # q/k tiles per batch (16)
    NQG = NQT // 4               # 512-col q-groups per batch (4)
    NCH = FLAT // RS_ROWS        # ReduceScatter chunks (8)
    SCALE = 1.0 / math.sqrt(HD)
    rg = [list(range(N_CORES))]

    nc = bacc.Bacc("TRN2", target_bir_lowering=False, debug=False,
                   num_devices=N_CORES)

    # ---- external parameters (partition-major for big-descriptor DMAs) ----
    xch_d = nc.declare_dram_parameter("xch", [NSC, 128, DT * SC_W], bf16,
                                      isOutput=False)
    wqp_d = nc.declare_dram_parameter("wqp", [128, DT * OC], bf16, isOutput=False)
    wkp_d = nc.declare_dram_parameter("wkp", [128, DT * OC], bf16, isOutput=False)
    wvp_d = nc.declare_dram_parameter("wvp", [128, DT * OC], bf16, isOutput=False)
    wop_d = nc.declare_dram_parameter("wop", [128, HPC * DIM], bf16, isOutput=False)
    cos_d = nc.declare_dram_parameter("cos_t", [HD, S], bf16, isOutput=False)
    sin_d = nc.declare_dram_parameter("sin_t", [HD, S], bf16, isOutput=False)
    mdg_d = nc.declare_dram_parameter("mdgp", [128, NQT * 128], fp32, isOutput=False)
    rot_d = nc.declare_dram_parameter("rotp", [128, 128], bf16, isOutput=False)
    one_d = nc.declare_dram_parameter("ones_bf", [128, 1], bf16, isOutput=False)
    onr_d = nc.declare_dram_parameter("ones_row", [1, 128], bf16, isOutput=False)
    out_d = nc.declare_dram_parameter("out", [FLAT // N_CORES, DIM], bf16,
                                      isOutput=True)

    # ---- internal DRAM (o-proj partials + RS outputs) ----
    par_d = [nc.dram_tensor(f"partial_dram{c}", [RS_ROWS, DIM], bf16)
             for c in range(NCH)]
    rs_d = [nc.dram_tensor(f"rs_out{c}", [RS_ROWS // N_CORES, DIM], bf16)
            for c in range(NCH)]

    from contextlib import ExitStack
    with tile.TileContext(nc) as tc:
        with ExitStack() as _stk:
            cpool = _stk.enter_context(tc.tile_pool(name="consts", bufs=1))
            qkvres = _stk.enter_context(tc.tile_pool(name="qkvres", bufs=1))
            xpool = _stk.enter_context(tc.tile_pool(name="xT", bufs=2))
            spool = _stk.enter_context(tc.tile_pool(name="cops", bufs=8))
            ptpool = _stk.enter_context(tc.tile_pool(name="probsT", bufs=5))
            accpool = _stk.enter_context(tc.tile_pool(name="accs", bufs=1))
            smpool = _stk.enter_context(tc.tile_pool(name="small", bufs=2))
            opool = _stk.enter_context(tc.tile_pool(name="outT", bufs=2))
            papool = _stk.enter_context(tc.tile_pool(name="partial", bufs=4))
            qkvps = _stk.enter_context(
                tc.tile_pool(name="qkvps", bufs=2, space="PSUM"))
            scps = _stk.enter_context(
                tc.tile_pool(name="scps", bufs=2, space="PSUM"))
            wkps = _stk.enter_context(
                tc.tile_pool(name="wkps", bufs=2, space="PSUM"))
            pops = _stk.enter_context(
                tc.tile_pool(name="pops", bufs=1, space="PSUM"))

            # ---- consts (gpsimd queue) ----
            wot_sb = cpool.tile([128, HPC, DIM], bf16)
            nc.gpsimd.dma_start(
                wot_sb[:], wop_d[:].rearrange("p (h e) -> p h e", h=HPC))
            cos_sb = cpool.tile([HD, S], bf16)
            nc.gpsimd.dma_start(cos_sb[:], cos_d[:])
            sin_sb = cpool.tile([HD, S], bf16)
            nc.gpsimd.dma_start(sin_sb[:], sin_d[:])
            mdg_sb = cpool.tile([128, NQT, 128], fp32)
            nc.gpsimd.dma_start(
                mdg_sb[:], mdg_d[:].rearrange("p (t k) -> p t k", t=NQT))
            rot_sb = cpool.tile([128, 128], bf16)
            nc.gpsimd.dma_start(rot_sb[:], rot_d[:])
            one_sb = cpool.tile([128, 1], bf16)
            nc.gpsimd.dma_start(one_sb[:], one_d[:])
            onr_sb = cpool.tile([1, 128], bf16)
            nc.gpsimd.dma_start(onr_sb[:], onr_d[:])

            # ---- weights + x chunks: one partition-major DMA each ----
            w_sb = {}
            xts = {}

            def load_x(sc, split=1):
                xt = xpool.tile([128, DT, SC_W], bf16, tag="xt", name=f"xt{sc}")
                step = DT // split
                for j in range(split):
                    nc.sync.dma_start(
                        xt[:, j * step:(j + 1) * step, :],
                        xch_d[sc, :, j * step * SC_W:(j + 1) * step * SC_W]
                        .rearrange("p (t c) -> p t c", t=step))
                xts[sc] = xt

            # prologue: both DGE queues, finest pieces first, in the order
            # the chains consume them (v chains first, then q, then k).
            def w_piece(nm, src, j, step, eng):
                eng.dma_start(
                    w_sb[nm][:, j * step:(j + 1) * step, :],
                    src[:, j * step * OC:(j + 1) * step * OC]
                    .rearrange("p (t e) -> p t e", t=step))

            for nm, src in (("v", wvp_d), ("q", wqp_d), ("k", wkp_d)):
                w_sb[nm] = qkvres.tile([128, DT, OC], bf16, tag=f"w{nm}",
                                       name=f"w{nm}")
            # sync queue: wv + x0 interleaved, then x1
            w_piece("v", wvp_d, 0, 4, nc.sync)
            xt0 = xpool.tile([128, DT, SC_W], bf16, tag="xt", name="xt0")
            xts[0] = xt0
            for j in range(4):
                nc.sync.dma_start(
                    xt0[:, j * 4:(j + 1) * 4, :],
                    xch_d[0, :, j * 4 * SC_W:(j + 1) * 4 * SC_W]
                    .rearrange("p (t c) -> p t c", t=4))
                if j < 3:
                    w_piece("v", wvp_d, j + 1, 4, nc.sync)
            # scalar queue: wq then wk, quartered, consumption order
            for j in range(4):
                w_piece("q", wqp_d, j, 4, nc.scalar)
            for j in range(4):
                w_piece("k", wkp_d, j, 4, nc.scalar)

            # ---- SBUF-resident q/k/v per (batch, head) ----
            qT = {(b, h): qkvres.tile([128, S], bf16, tag=f"qT{b}{h}", name=f"qT{b}{h}")
                  for b in range(B) for h in range(HPC)}
            kT = {(b, h): qkvres.tile([128, S], bf16, tag=f"kT{b}{h}", name=f"kT{b}{h}")
                  for b in range(B) for h in range(HPC)}
            vN = {(b, h): qkvres.tile([128, NQT, HD], bf16, tag=f"vN{b}{h}",
                                      name=f"vN{b}{h}")
                  for b in range(B) for h in range(HPC)}

            oT_of = {}

            # =========== emission generators ===========

            def _emit_rope(bb, s0, t, h, til):
                rp = wkps.tile([128, SC_W], fp32, tag="wk", name=f"rot{t}{h}")
                nc.tensor.matmul(rp[:], rot_sb[:], til[:], start=True, stop=True)
                dst = qT[(bb, h)] if t == "q" else kT[(bb, h)]
                t1 = spool.tile([128, SC_W], bf16, tag="t1", name=f"t1{t}{h}")
                nc.vector.tensor_mul(t1[:], til[:], cos_sb[:, s0:s0 + SC_W])
                hat = spool.tile([128, SC_W], bf16, tag="hat", name=f"hat{t}{h}")
                nc.vector.tensor_mul(hat[:], rp[:], sin_sb[:, s0:s0 + SC_W])
                nc.vector.tensor_add(dst[:, s0:s0 + SC_W], hat[:], t1[:])

            def chunk_gen(sc):
                """QKV projections + RoPE for one schunk; yields per chain."""
                bb, c0 = divmod(sc * SC_W, S)
                s0 = c0
                chains = [(t, h) for t in ("v", "q", "k") for h in range(HPC)]
                pend = []
                for ci, (t, h) in enumerate(chains):
                    ps = qkvps.tile([128, SC_W], fp32, tag="qkv", name=f"ps_{t}{h}")
                    for dt in range(DT):
                        nc.tensor.matmul(
                            ps[:],
                            w_sb[t][:, dt, h * HD:(h + 1) * HD],
                            xts[sc][:, dt, :],
                            start=(dt == 0), stop=(dt == DT - 1))
                    if t in ("q", "k"):
                        til = spool.tile([128, SC_W], bf16, tag="til",
                                         name=f"til{t}{h}")
                        nc.scalar.activation(til[:], ps[:], Copy,
                                             scale=SCALE if t == "q" else 1.0)
                        pend.append((t, h, til))
                    else:
                        vb = spool.tile([128, SC_W], bf16, tag="vb", name=f"vb{h}")
                        nc.scalar.copy(vb[:], ps[:])
                        kt0 = c0 // 128
                        nc.sync.dma_start_transpose(
                            vN[(bb, h)][:, kt0:kt0 + 4, :], vb[:])
                    if len(pend) > 1:
                        _emit_rope(bb, s0, *pend.pop(0))
                    yield
                for args in pend:
                    _emit_rope(bb, s0, *args)

            def attn_gen(bb, qg):
                """Attention for 512 q-cols (group qg); yields per kt step."""
                kmax = qg * 4 + 3
                po = {h: pops.tile([128, 512], fp32, tag=f"po{h}", name=f"po{h}")
                      for h in range(HPC)}
                acc_v = {h: accpool.tile([128, 512], bf16, tag=f"av{h}",
                                         name=f"accv{h}") for h in range(HPC)}
                acc_g = {h: accpool.tile([128, 512], bf16, tag=f"ag{h}",
                                         name=f"accg{h}") for h in range(HPC)}
                pt_hist = {h: {} for h in range(HPC)}

                def rowsum(h, kt, qlo, n):
                    pt = pt_hist[h][kt]
                    if qg == 0:
                        if kt == 0:
                            nc.vector.tensor_copy(acc_v[h][:], pt[:, :n])
                        else:
                            nc.vector.tensor_add(acc_v[h][:, qlo:512],
                                                 acc_v[h][:, qlo:512], pt[:, :n])
                        return
                    if kt == 0:
                        nc.vector.tensor_copy(acc_v[h][:], pt[:, :n])
                    elif kt == 1:
                        nc.vector.tensor_copy(acc_g[h][:], pt[:, :n])
                    elif kt % 2 == 0:
                        nc.vector.tensor_add(acc_v[h][:, qlo:512],
                                             acc_v[h][:, qlo:512], pt[:, :n])
                    else:
                        nc.vector.tensor_add(acc_g[h][:, qlo:512],
                                             acc_g[h][:, qlo:512], pt[:, :n])

                for kt in range(kmax + 1):
                    qlo = max(0, kt - qg * 4) * 128
                    n = 512 - qlo
                    for h in range(HPC):
                        sp = scps.tile([128, 512], fp32, tag="sc", name="sp")
                        nc.tensor.matmul(
                            sp[:, :n],
                            kT[(bb, h)][:, kt * 128:(kt + 1) * 128],
                            qT[(bb, h)][:, qg * 512 + qlo:(qg + 1) * 512],
                            start=True, stop=True)
                        if kt >= qg * 4:  # diagonal block: causal mask
                            nc.vector.tensor_add(
                                sp[:, 0:128], sp[:, 0:128], mdg_sb[:, kt, :])
                        pt = ptpool.tile([128, 512], bf16, tag=f"pT{h}",
                                         name=f"pT{h}")
                        pt_hist[h][kt] = pt
                        nc.scalar.activation(pt[:, :n], sp[:, :n], Exp)
                        rowsum(h, kt, qlo, n)
                    if kt >= 1:
                        kl = kt - 1
                        ql2 = max(0, kl - qg * 4) * 128
                        n2 = 512 - ql2
                        for h in range(HPC):
                            nc.tensor.matmul(
                                po[h][:, ql2:512], vN[(bb, h)][:, kl, :],
                                pt_hist[h][kl][:, :n2],
                                start=(kl == 0), stop=False)
                    yield
                for h in range(HPC):
                    nc.tensor.matmul(
                        po[h][:, 384:512], vN[(bb, h)][:, kmax, :],
                        pt_hist[h][kmax][:, :128], start=False, stop=True)

                # softmax denominators: two cheap PE matmuls (partition
                # reduce via ones, then partition broadcast) -- gpsimd
                # stays dedicated to collectives so a blocking RS trigger
                # can't delay the normalize chain.
                oT_of[(bb, qg)] = {}
                for h in range(HPC):
                    if qg > 0:
                        nc.vector.tensor_add(acc_v[h][:], acc_v[h][:],
                                             acc_g[h][:])
                    srow_ps = scps.tile([1, 512], fp32, tag="sc", name="srow_ps")
                    nc.tensor.matmul(srow_ps[:], one_sb[:], acc_v[h][:],
                                     start=True, stop=True)
                    srow = smpool.tile([1, 512], bf16, tag="srow", name="srow")
                    nc.scalar.copy(srow[:], srow_ps[:])
                    sbc_ps = scps.tile([128, 512], fp32, tag="sc", name="sbc_ps")
                    nc.tensor.matmul(sbc_ps[:], onr_sb[:], srow[:],
                                     start=True, stop=True)
                    rbc = smpool.tile([128, 512], fp32, tag="rbc", name="rbc")
                    nc.vector.reciprocal_approx_fast(rbc[:], sbc_ps[:])
                    ot = opool.tile([128, 512], bf16, tag=f"oT{h}", name=f"oT{h}")
                    nc.vector.tensor_mul(ot[:], po[h][:], rbc[:])
                    oT_of[(bb, qg)][h] = ot

            def oproj_gen(bb, qg):
                """O-projection + ReduceScatter for one unit; yields per st."""
                ot = oT_of.pop((bb, qg))
                chx = bb * NQG + qg
                for st in range(4):
                    par = papool.tile([128, DIM], bf16, tag="par", name="par")
                    for ec in range(4):
                        pp = wkps.tile([128, 512], fp32, tag="wk", name="pp")
                        for h in range(HPC):
                            nc.tensor.matmul(
                                pp[:],
                                ot[h][:, st * 128:(st + 1) * 128],
                                wot_sb[:, h, ec * 512:(ec + 1) * 512],
                                start=(h == 0), stop=(h == HPC - 1))
                        if ec % 2 == 0:
                            nc.scalar.copy(par[:, ec * 512:(ec + 1) * 512], pp[:])
                        else:
                            nc.vector.tensor_copy(
                                par[:, ec * 512:(ec + 1) * 512], pp[:])
                    nc.sync.dma_start(par_d[chx][st * 128:(st + 1) * 128, :],
                                      par[:])
                    if st < 3:
                        yield
                nc.gpsimd.collective_compute(
                    "ReduceScatter", ADD, replica_groups=rg,
                    ins=[par_d[chx][:]], outs=[rs_d[chx][:]])

            def pump(gen, n):
                """Advance gen up to n steps; return True when exhausted."""
                if gen is None:
                    return True
                for _ in range(n):
                    if next(gen, _DONE) is _DONE:
                        return True
                return False

            _DONE = object()

            def drain(ag, ogs):
                a_done = ag is None
                ogs = list(ogs)
                while not (a_done and not ogs):
                    if not a_done:
                        a_done = pump(ag, 2)
                    for og in list(ogs):
                        if pump(og, 1):
                            ogs.remove(og)

            # =========== main schedule ===========
            # iteration sc: chunk(sc) ⊗ attn(unit sc-1) ⊗ oproj(unit sc-2)
            units = [(s // NQG, s % NQG) for s in range(NSC)]
            for sc in range(NSC):
                if sc + 1 < NSC:
                    load_x(sc + 1)
                cg = chunk_gen(sc)
                ag = attn_gen(*units[sc - 1]) if sc >= 1 else None
                ogs = [oproj_gen(*units[sc - 2])] if sc >= 2 else []
                a_steps = (units[sc - 1][1] * 4 + 4) if ag else 0
                done_a = ag is None
                for ci in range(6):
                    pump(cg, 1)
                    for og in list(ogs):
                        if ci >= 1 and pump(og, 1):
                            ogs.remove(og)
                    if not done_a:
                        share = max(1, -(-a_steps // 6))
                        done_a = pump(ag, share)
                pump(cg, 1)  # run the trailing rope flush
                drain(None if done_a else ag, ogs)
            # post-loop: attn of last unit ⊗ oproj of second-to-last
            drain(attn_gen(*units[NSC - 1]), [oproj_gen(*units[NSC - 2])])
            drain(None, [oproj_gen(*units[NSC - 1])])
            # rs -> out copies: chunks 0..6 long done (no queue blocking);
            # chunk 7's copy rides the RS tail.
            for chx in range(NCH):
                nc.gpsimd.dma_start(out_d[chx * 64:(chx + 1) * 64, :],
                                    rs_d[chx][:])

    nc.compile()
    return nc


def _get_nc(S):
    if S not in _CACHE:
        _CACHE[S] = _build(S)
    return _CACHE[S]


def make_inputs(x, freqs_cis, mask, wq, wk, wv, wo):
    """Host-side sharding / layout prep. Returns in_maps for 8 cores."""
    import ml_dtypes
    bf = ml_dtypes.bfloat16
    S = x.shape[1]
    FLAT = B * S
    NSC = FLAT // SC_W

    flat_xt = np.asarray(x, np.float32).reshape(FLAT, DIM).T  # [DIM, FLAT]
    # chunk-partition-major: xch[sc, p, t*SC_W + c] = xt[t*128+p, sc*SC_W+c]
    xch = np.ascontiguousarray(
        flat_xt.reshape(DT, 128, FLAT)[:, :, :]
        .transpose(1, 0, 2)            # [128, DT, FLAT]
        .reshape(128, DT, NSC, SC_W)
        .transpose(2, 0, 1, 3)         # [NSC, 128, DT, SC_W]
        .reshape(NSC, 128, DT * SC_W)).astype(bf)

    cos = np.asarray(freqs_cis[..., 0], np.float32)   # [S, HD/2]
    sin = np.asarray(freqs_cis[..., 1], np.float32)
    cos_t = np.ascontiguousarray(np.repeat(cos.T, 2, axis=0)).astype(bf)
    sin_t = np.ascontiguousarray(np.repeat(sin.T, 2, axis=0)).astype(bf)
    m = np.asarray(mask, np.float32)[0, 0]
    nqt = S // 128
    mask_diag = np.stack([m[i * 128:(i + 1) * 128, i * 128:(i + 1) * 128].T
                          for i in range(nqt)])       # [NQT, 128, 128]
    mdgp = np.ascontiguousarray(
        mask_diag.transpose(1, 0, 2).reshape(128, nqt * 128))

    P = np.zeros((128, 128), np.float32)
    for j in range(64):
        P[2 * j, 2 * j + 1] = -1.0
        P[2 * j + 1, 2 * j] = 1.0
    rotp = np.ascontiguousarray(P.T).astype(bf)

    def pmajor(w):  # [DIM, OC] -> [128, DT*OC]
        return np.ascontiguousarray(
            w.reshape(DT, 128, -1).transpose(1, 0, 2).reshape(128, -1))

    in_maps = []
    for c in range(N_CORES):
        r = slice(c * OC, (c + 1) * OC)
        wqt = np.asarray(wq, np.float32)[r, :].T      # [DIM, OC]
        wkt = np.asarray(wk, np.float32)[r, :].T
        wvt = np.asarray(wv, np.float32)[r, :].T
        wot = np.asarray(wo, np.float32)[:, r].T      # [OC, DIM]
        wop = np.ascontiguousarray(
            wot.reshape(HPC, 128, DIM).transpose(1, 0, 2).reshape(128, -1))
        in_maps.append({
            "xch": xch,
            "wqp": pmajor(wqt).astype(bf),
            "wkp": pmajor(wkt).astype(bf),
            "wvp": pmajor(wvt).astype(bf),
            "wop": wop.astype(bf),
            "cos_t": cos_t,
            "sin_t": sin_t,
            "mdgp": mdgp,
            "rotp": rotp,
            "ones_bf": np.ones((128, 1), dtype=bf),
            "ones_row": np.ones((1, 128), dtype=bf),
        })
    return in_maps


def assemble(results, S):
    """Undo the per-core ReduceScatter sharding into the full output."""
    nch = B * S // RS_ROWS
    full = np.empty((B * S, DIM), np.float32)
    for c in range(N_CORES):
        o = np.asarray(results[c]["out"], np.float32)  # [512, DIM]
        for chx in range(nch):
            full[chx * 512 + c * 64:chx * 512 + (c + 1) * 64] = \
                o[chx * 64:(chx + 1) * 64]
    return full.reshape(B, S, DIM)


def kernel(x, start_pos, freqs_cis, mask, wq, wk, wv, wo):
    from concourse.bass_utils import run_bass_kernel_spmd
    S = x.shape[1]
    nc = _get_nc(S)
    in_maps = make_inputs(x, freqs_cis, mask, wq, wk, wv, wo)
    res = run_bass_kernel_spmd(nc, in_maps, core_ids=list(range(N_CORES)))
    return assemble(res.results, S)
